# revision 17
# baseline (speedup 1.0000x reference)
"""Trainium2 Bass kernel for nn_ModelNew_3556232922055 (dense_cnn).

Semantics (per image):
  y8[j]    = conv2d_valid(x, weight[:8]) + bias[:8]          (8,126,126)
  acc[co]  = max over (ci,kh,kw) of 2*W[co,ci,kh,kw]*y8[ci,h+kh,w+kw]
             (out-of-range taps excluded at the bottom/right borders)
  out      = min over co of acc                              (1,126,126)

Sharding: data-parallel over batch, 1 image per NeuronCore (8 cores).

Device mapping per core:
  - im2col X72 [72, 16128] f32 built by one DMA from DRAM (overlapped reads)
  - conv as a single k=72 matmul pass -> PSUM -> ACT evac (+bias) into
    Y8REP [128, 16128] where partition p = ci*16 + r holds y8[ci] (16 replicas)
  - step 2: two passes (co 0-15, co 16-31); pacc[p = ci*16+co] accumulates
    max over the 9 taps of scal[p,t]*y8[ci, pix+off_t] using
    tensor_scalar(mult) products + tensor_tensor(max) accumulation on DVE,
    with a configurable subset of the products computed on ACT instead.
  - reduce: PE-transpose 128x128 chunks -> ACT evac -> DVE max-fold over ci,
    min-fold over co -> OUT[w,h] -> PE transpose -> DMA out (126,126).
"""

import numpy as np
from contextlib import ExitStack

import concourse.bass as bass
import concourse.tile as tile
from concourse import bacc, mybir
from concourse import masks
from concourse.bass_utils import run_bass_kernel_spmd

F32 = mybir.dt.float32
BF16 = mybir.dt.bfloat16

# dtype of y8 replicas and pacc accumulators (bf16 = fast path)
DT_Y = BF16
DT_ACC = BF16
# tap indices (1..8, i.e. 9 taps per pass excluding tap 0) whose product
# is computed on the scalar (ACT) engine instead of DVE, per pass.
# Tuned so ACT time ~ DVE time.
ACT_TAPS = (1, 2, 3, 4, 5)

H = W = 128
CIN = 8
COUT = 32
K = 3
OH = OW = 126
NPIX = H * OH          # 16128 flat pixels (h*128+w), h<126
NCORES = 8
CHUNK = 504            # conv free-dim chunk (<=512, 32*504=16128)
GROUP = 16             # reduce-phase chunks per group (126 = 7*16 + 14)

TAPS = [(kh, kw) for kh in range(K) for kw in range(K)]


def _r3(t, h0, w0, nh, nw):
    """3D region view [128, nh, nw] of a [128, NPIX] tile at (h0, w0)."""
    return t[:].rearrange("p (h w) -> p h w", w=W)[:, h0 : h0 + nh, w0 : w0 + nw]


def build_program():
    nc = bacc.Bacc()

    # x72: host-built im2col, x72[(kh*3+kw)*8+ci, pix] = x[ci, pix+kh*128+kw]
    x_d = nc.declare_dram_parameter("x72", [72, NPIX], F32, isOutput=False)
    # consts: [:, 0:18] scal, [:, 18:19] bias128, [0:72, 19:147] w1rep
    c_d = nc.declare_dram_parameter("consts", [128, 147], F32, isOutput=False)
    out_d = nc.declare_dram_parameter("out", [OH, OW], F32, isOutput=True)

    with ExitStack() as ctx:
        tc = ctx.enter_context(tile.TileContext(nc))

        consts = ctx.enter_context(tc.tile_pool(name="consts", bufs=1))
        big = ctx.enter_context(tc.tile_pool(name="big", bufs=1))

        constst = consts.tile([128, 147], F32)
        nc.sync.dma_start(constst[:], c_d[:])
        scalt = constst[:, 0:18]
        biast = constst[:, 18:19]
        w1t = constst[0:72, 19:147]
        ident = consts.tile([128, 128], DT_ACC)
        masks.make_identity(nc, ident[:])
        ident_f32 = consts.tile([128, 128], F32)
        masks.make_identity(nc, ident_f32[:])

        y8 = big.tile([128, NPIX], DT_Y)
        with tc.tile_pool(name="xp", bufs=1) as xp:
            x72 = xp.tile([72, NPIX], F32)
            nc.sync.dma_start(out=x72[:], in_=x_d[:])

            # --- conv: y8rep[p = ci*16+r] = y8[ci], bf16 ---
            with tc.tile_pool(name="convps", bufs=2, space="PSUM") as convps:
                for c in range(NPIX // CHUNK):
                    n0 = c * CHUNK
                    ps = convps.tile([128, CHUNK], F32)
                    nc.tensor.matmul(
                        ps[:], lhsT=w1t[:], rhs=x72[:, n0 : n0 + CHUNK],
                        start=True, stop=True,
                    )
                    nc.scalar.activation(
                        y8[:, n0 : n0 + CHUNK], ps[:],
                        mybir.ActivationFunctionType.Identity,
                        bias=biast[:, 0:1], scale=1.0,
                    )

        ppool = ctx.enter_context(tc.tile_pool(name="ppool", bufs=2))
        redpool = ctx.enter_context(tc.tile_pool(name="redpool", bufs=2))
        outpool = ctx.enter_context(tc.tile_pool(name="outpool", bufs=1))

        # --- step 2: pacc[p = ci*16 + co_lo] = max_t scal[p,t]*y8[ci, pix+off] ---
        paccs = []
        for half in range(2):
            pacc = big.tile([128, NPIX], DT_ACC, tag=f"pacc{half}")
            paccs.append(pacc)
            sc = scalt[:, half * 9 : half * 9 + 9]
            # tap (0,0): covers every cell read later (h<126, all 128 w)
            nc.vector.tensor_scalar(
                _r3(pacc, 0, 0, OH, W), _r3(y8, 0, 0, OH, W),
                sc[:, 0:1], None, mybir.AluOpType.mult,
            )
            for t in range(1, 9):
                kh, kw = TAPS[t]
                nh, nw = OH - kh, OW - kw
                p = ppool.tile([128, NPIX], DT_ACC, tag="P")
                if t in ACT_TAPS:
                    nc.scalar.mul(_r3(p, 0, 0, nh, nw), _r3(y8, kh, kw, nh, nw),
                                  sc[:, t : t + 1])
                else:
                    nc.vector.tensor_scalar(
                        _r3(p, 0, 0, nh, nw), _r3(y8, kh, kw, nh, nw),
                        sc[:, t : t + 1], None, mybir.AluOpType.mult,
                    )
                nc.vector.tensor_tensor(
                    _r3(pacc, 0, 0, nh, nw), _r3(pacc, 0, 0, nh, nw),
                    _r3(p, 0, 0, nh, nw), mybir.AluOpType.max,
                )

        # --- reduce: max over ci (8 partition-groups), min over 32 co ---
        outt = outpool.tile([128, OH], F32)  # OUT[w, h]
        with tc.tile_pool(name="redps", bufs=1, space="PSUM") as redps:
            ngroups = (OH + GROUP - 1) // GROUP
            for g in range(ngroups):
                c0 = g * GROUP
                gc = min(GROUP, OH - c0)
                ps_a = redps.tile([128, gc * 128], DT_ACC, tag="ps_a")
                ps_b = redps.tile([128, gc * 128], DT_ACC, tag="ps_b")
                pst = [ps_a, ps_b]
                for half in range(2):
                    for j in range(gc):
                        nc.tensor.transpose(
                            pst[half][:, j * 128 : (j + 1) * 128],
                            paccs[half][:, (c0 + j) * 128 : (c0 + j + 1) * 128],
                            ident[:],
                        )
                pt = redpool.tile([128, 2 * gc * 128], DT_ACC, tag="PT")
                for half in range(2):
                    nc.scalar.copy(
                        pt[:, half * gc * 128 : (half + 1) * gc * 128], pst[half][:]
                    )
                # pt layout: [p][s=2][c=gc][ci=8][co=16]
                v = pt[:].rearrange("p (s c ci co) -> p s c ci co", s=2, c=gc, ci=8)
                mx = mybir.AluOpType.max
                mn = mybir.AluOpType.min
                nc.vector.tensor_tensor(
                    v[:, :, :, 0:4, :], v[:, :, :, 0:4, :], v[:, :, :, 4:8, :], mx)
                nc.vector.tensor_tensor(
                    v[:, :, :, 0:2, :], v[:, :, :, 0:2, :], v[:, :, :, 2:4, :], mx)
                nc.vector.tensor_tensor(
                    v[:, :, :, 0:1, :], v[:, :, :, 0:1, :], v[:, :, :, 1:2, :], mx)
                # min(A-half, B-half) -> co 0..15 vs 16..31
                nc.vector.tensor_tensor(
                    v[:, 0:1, :, 0:1, :], v[:, 0:1, :, 0:1, :], v[:, 1:2, :, 0:1, :], mn)
                w2 = v[:, 0, :, 0, :]  # [p, c, co16]
                nc.vector.tensor_tensor(w2[:, :, 0:8], w2[:, :, 0:8], w2[:, :, 8:16], mn)
                nc.vector.tensor_tensor(w2[:, :, 0:4], w2[:, :, 0:4], w2[:, :, 4:8], mn)
                nc.vector.tensor_tensor(w2[:, :, 0:2], w2[:, :, 0:2], w2[:, :, 2:4], mn)
                nc.vector.tensor_tensor(
                    outt[:, c0 : c0 + gc], w2[:, :, 0:1], w2[:, :, 1:2], mn)

            # transpose OUT[w,h] -> [h,w] and write out
            pso = redps.tile([128, 128], F32, tag="pso")
            nc.tensor.transpose(pso[0:OH, :], outt[:, 0:OH], ident_f32[:])
            res = outpool.tile([128, 128], F32)
            nc.scalar.copy(res[0:OH, :], pso[0:OH, :])
            nc.sync.dma_start(out_d[:, :], res[0:OH, 0:OW])

    nc.compile()
    return nc


def host_tiles(weight, bias):
    weight = np.asarray(weight, np.float32)
    bias = np.asarray(bias, np.float32)
    w1rep = np.zeros((72, 128), np.float32)
    for kh in range(K):
        for kw in range(K):
            for ci_in in range(CIN):
                t = (kh * K + kw) * CIN + ci_in
                for ci_out in range(CIN):
                    w1rep[t, ci_out * 16 : ci_out * 16 + 16] = weight[
                        ci_out, ci_in, kh, kw
                    ]
    bias128 = np.repeat(bias[:CIN], 16).astype(np.float32).reshape(128, 1)
    scal = np.zeros((128, 18), np.float32)
    for p in range(128):
        ci = p // 16
        co_lo = p % 16
        for half in range(2):
            co = co_lo + 16 * half
            for t, (kh, kw) in enumerate(TAPS):
                scal[p, half * 9 + t] = 2.0 * weight[co, ci, kh, kw]
    consts = np.zeros((128, 147), np.float32)
    consts[:, 0:18] = scal
    consts[:, 18:19] = bias128
    consts[0:72, 19:147] = w1rep
    return consts


def im2col_host(xb):
    """xb: (8,128,128) f32 -> (72, NPIX) f32 with junk tail cols zeroed."""
    x72 = np.zeros((72, NPIX), np.float32)
    flat = xb.reshape(CIN, H * W)
    L = NPIX - 2
    for kh in range(K):
        for kw in range(K):
            for ci in range(CIN):
                t = (kh * K + kw) * CIN + ci
                off = kh * W + kw
                x72[t, :L] = xb.reshape(-1)[ci * H * W + off : ci * H * W + off + L]
    return x72


_CACHE = {}


def _get_program():
    if "nc" not in _CACHE:
        _CACHE["nc"] = build_program()
    return _CACHE["nc"]


def run_spmd(x, weight, bias, **kw):
    x = np.ascontiguousarray(np.asarray(x, np.float32))
    consts = host_tiles(weight, bias)
    nc = _get_program()
    in_maps = [
        {"x72": im2col_host(x[b]), "consts": consts} for b in range(NCORES)
    ]
    res = run_bass_kernel_spmd(nc, in_maps, list(range(NCORES)), **kw)
    out = np.stack([res.results[b]["out"] for b in range(NCORES)])
    return out[:, None, :, :].astype(np.float32), res


def kernel(x, weight, bias):
    out, _ = run_spmd(x, weight, bias)
    return out


if __name__ == "__main__":
    rng = np.random.default_rng(0)
    x = rng.standard_normal((8, CIN, H, W), dtype=np.float32)
    wt = rng.uniform(-0.1, 0.1, (COUT, CIN, K, K)).astype(np.float32)
    bs = rng.uniform(-0.1, 0.1, COUT).astype(np.float32)
    print(kernel(x, wt, bs).shape)
